# revision 17
# baseline (speedup 1.0000x reference)
"""Trainium2 Bass kernel for nn_DAC_structure: two-branch patch attention
(softmax(QK^T/sqrt(E)) -> channel mean -> repeat/tile expansion).

Sharding: data-parallel over the outer batch factor (8 cores x 32 channels).

v5 design (DMA-roofline oriented; the cost model serializes all DMAs on one
aggregate-bandwidth resource, so the schedule keeps that resource busy):
  - queries in fp8-e4m3 (matmul weights; pn fully, ps for half the channels
    -- quantization error averages across the channel mean, measured rel
    1.4e-2 vs the 2e-2 budget), keys bf16. Input bytes 1.69MB/core.
  - per-chunk PSUM/SBUF tiles: the tile dep tracker collapses many-writer
    tiles to whole-tile granularity, so shared tiles serialize chunk
    pipelines. One PSUM-bank-sized tile per ps k-chunk [1,3,4,4,2,2]; the
    tiny first chunk starts the softmax lattice early, small last chunks
    keep the post-input critical chain short.
  - pn branch loads first and computes while ps streams in; channel means
    via PSUM-accumulating selector matmuls (each accumulation group's
    matmuls kept contiguous -- the scheduler may otherwise reorder a
    start=True overwrite past an accumulate).
  - ps softmax: ACT exp -> DVE rowsum+recip -> DVE/Pool multiply; engine
    assignment balances the saturated DVE against Pool/ACT slack.
  - outputs: 2 pn half-head DMAs + 4 ps head-pair DMAs (SP queue, ordered
    by readiness); the 8x row expansion is done by the output DMA itself
    (stride-0 source AP) so output HBM traffic is the only cost; outputs
    bf16, upcast on host.
"""
import sys
import functools

for _p in ('/opt/trn_rl_repo', '/root/.axon_site/_ro/trn_rl_repo'):
    if _p not in sys.path:
        sys.path.append(_p)

import numpy as np
import concourse.bass as bass
import concourse.bacc as bacc
import concourse.tile as tile
from concourse import mybir

F32 = mybir.dt.float32
BF16 = mybir.dt.bfloat16
FP8 = mybir.dt.float8e4

CH = 32      # channels per core
H = 8        # heads
E = 64       # head dim
WIN = 256
N_CORES = 8
SCALE = 0.125          # 1/sqrt(E)
INV_CH = 1.0 / 32.0

# ps k-chunk boundaries; sizes are PSUM-bank aligned (<=2KB each).
# Tiny first chunk starts the ACT/DVE softmax lattice ~2us earlier;
# small last chunks keep the post-input critical chain short.
CHUNKS = [(0, 1), (1, 4), (4, 8), (8, 12), (12, 14), (14, 16)]


def _body(tc, qt_ps, qt_ps2, kt_ps, qt_pn, kt_pn, out_ps_dev, out_pn_dev):
    nc = tc.nc
    pools = [tc.tile_pool(name="sb", bufs=1),
             tc.tile_pool(name="ps", bufs=1, space="PSUM")]
    sb, psum = (p.__enter__() for p in pools)
    lowp = nc.allow_low_precision(reason="bf16/fp8 softmax, 2e-2 rel budget")
    lowp.__enter__()

    # ---- SBUF tiles ----
    # queries mixed precision: cp0-7 fp8, cp8-15 bf16 -- halves the
    # fp8 quantization contribution to the channel-mean (margin vs 2e-2)
    qs_ps = sb.tile([128, 8, 8, 32], FP8, tag="qs_ps")
    qs_ps2 = sb.tile([128, 8, 8, 32], BF16, tag="qs_ps2")
    ks_ps = sb.tile([128, 16, 8, 32], BF16, tag="ks_ps")
    qs_pn = sb.tile([128, 16, 8, 16], FP8, tag="qs_pn")
    ks_pn = sb.tile([128, 16, 8, 8], BF16, tag="ks_pn")
    p_c, pd_c, rs_c, rr_c = [], [], [], []
    for ci, (a, b) in enumerate(CHUNKS):
        n = b - a
        p_c.append(sb.tile([128, 4, n, 32], BF16, tag=f"p{ci}", name=f"p{ci}"))
        pd_c.append(sb.tile([128, 4, n, 32], BF16, tag=f"pd{ci}",
                            name=f"pd{ci}"))
        rs_c.append(sb.tile([128, 4, n], F32, tag=f"rs{ci}",
                            name=f"rs{ci}"))
        rr_c.append(sb.tile([128, 4, n], BF16, tag=f"rr{ci}",
                            name=f"rr{ci}"))
    p_pn = sb.tile([128, 2, 16, 8], BF16, tag="p_pn")
    pd_pn = sb.tile([128, 2, 16, 8], BF16, tag="pd_pn")
    rs_pn = sb.tile([128, 2, 16], F32, tag="rs_pn")
    rr_pn = sb.tile([128, 2, 16], BF16, tag="rr_pn")
    e_ps = [sb.tile([128, 32, 8], BF16, tag=f"e_ps{g}", name=f"e_ps{g}")
            for g in range(2)]
    e_pn = sb.tile([64, 32, 8], BF16, tag="e_pn")

    # ---- input DMAs ----
    # kpnA goes out on the ACT queue so it interleaves with SP's first two
    # issues (SP alone spaces DMAs ~650ns apart and starves the DMA engines
    # on short transfers); either arrival order lands pn inputs by ~3.1us.
    nc.sync.dma_start(out=qs_pn[:, :, :, :], in_=qt_pn[:, :, :, :])
    nc.sync.dma_start(out=ks_pn[:, :, :, :], in_=kt_pn[:, :, :, :])
    nc.sync.dma_start(out=qs_ps[:, :, :, :], in_=qt_ps[:, :, :, :])
    nc.sync.dma_start(out=ks_ps[:, 0:1, :, :], in_=kt_ps[:, 0:1, :, :])
    nc.sync.dma_start(out=ks_ps[:, 1:4, :, :], in_=kt_ps[:, 1:4, :, :])
    nc.sync.dma_start(out=ks_ps[:, 4:8, :, :], in_=kt_ps[:, 4:8, :, :])
    nc.sync.dma_start(out=qs_ps2[:, :, :, :], in_=qt_ps2[:, :, :, :])
    nc.sync.dma_start(out=ks_ps[:, 8:12, :, :], in_=kt_ps[:, 8:12, :, :])
    nc.sync.dma_start(out=ks_ps[:, 12:16, :, :], in_=kt_ps[:, 12:16, :, :])

    # ---- selector constants (Pool engine; no input deps) ----
    sel_ps = sb.tile([128, 64], BF16, tag="sel_ps")
    nc.gpsimd.memset(sel_ps[:, :], 0.0)
    for cA in range(2):
        nc.gpsimd.affine_select(
            out=sel_ps[:, :], in_=sel_ps[:, :],
            compare_op=mybir.AluOpType.not_equal, fill=INV_CH,
            base=-32 * cA, pattern=[[-64, 2], [-1, 32]], channel_multiplier=1)
    sel_pn = sb.tile([128, 64], BF16, tag="sel_pn")
    nc.gpsimd.memset(sel_pn[:, :], 0.0)
    for cA in range(2):
        nc.gpsimd.affine_select(
            out=sel_pn[:, :], in_=sel_pn[:, :],
            compare_op=mybir.AluOpType.not_equal, fill=INV_CH,
            base=-8 * cA, pattern=[[0, 2], [-32, 4], [-1, 8]],
            channel_multiplier=1)

    # ---- PSUM tiles ----
    big = []
    for ci, (a, b) in enumerate(CHUNKS):
        big.append(psum.tile([128, 4, b - a, 32], F32, tag=f"big{ci}",
                             name=f"big{ci}"))
    bank_pn = psum.tile([128, 2, 16, 8], F32, tag="bank_pn",
                        name="bank_pn")
    # one PSUM bank holds both mean accumulators: ps means in [:, :, 0:32],
    # pn means in [0:64, 0, 32:40] (bank-granular allocator, only 8 banks)
    m_all = psum.tile([128, 2, 40], F32, tag="m_all", name="m_all")
    # unused pn bank rows must be 0 (exp(0)=1 killed by sel weights)
    nc.vector.memset(bank_pn[:, :, :, :], 0.0)

    # ---- PE: pn scores ----
    for cp in range(16):
        for h in range(8):
            pb, hh = h % 4, h // 4
            nc.tensor.matmul(
                bank_pn[pb * 32:pb * 32 + 16, hh, cp, :],
                qs_pn[:, cp, h, :], ks_pn[:, cp, h, :],
                start=True, stop=True, tile_position=(0, pb * 32))

    # ---- pn softmax (ACT exp, DVE rowsum+recip, Pool mult) ----
    nc.scalar.activation(out=p_pn[:, :, :, :], in_=bank_pn[:, :, :, :],
                         func=mybir.ActivationFunctionType.Exp, scale=SCALE)
    nc.vector.reduce_sum(out=rs_pn[:, :, :], in_=p_pn[:, :, :, :],
                         axis=mybir.AxisListType.X)
    nc.vector.reciprocal(out=rr_pn[:, :, :], in_=rs_pn[:, :, :])
    nc.gpsimd.tensor_tensor(
        out=pd_pn[:, :, :, :], in0=p_pn[:, :, :, :],
        in1=rr_pn[:, :, :].unsqueeze(3).broadcast_to([128, 2, 16, 8]),
        op=mybir.AluOpType.mult)

    # ---- PE: ps score chunks ----
    def ps_scores(ci):
        a, b = CHUNKS[ci]
        ofs = a
        for cp in range(a, b):
            for h in range(8):
                hp, hs = divmod(h, 2)
                for cA in range(2):
                    r0 = cA * 64
                    qv = (qs_ps[r0:r0 + 64, cp, h, :] if cp < 8 else
                          qs_ps2[r0:r0 + 64, cp - 8, h, :])
                    nc.tensor.matmul(
                        big[ci][hs * 64 + cA * 32:hs * 64 + cA * 32 + 32,
                                hp, cp - ofs, :],
                        qv,
                        ks_ps[r0:r0 + 64, cp, h, :],
                        start=True, stop=True,
                        tile_position=(cA * 64, hs * 64 + cA * 32))

    # ---- PE: pn channel-mean (PSUM-accumulated selector matmuls) ----
    def pn_sel(a, b):
        for hh in range(2):
            for cp in range(a, b):
                nc.tensor.matmul(
                    m_all[hh * 32:hh * 32 + 32, 0, 32:40], sel_pn[:, 0:32],
                    pd_pn[:, hh, cp, :],
                    start=(cp == 0), stop=(cp == 15),
                    tile_position=(0, hh * 32))

    ps_scores(0)
    ps_scores(1)
    pn_sel(0, 16)

    # ---- pn expand on DVE (Pool cannot read PSUM) + out DMAs (SP) ----
    # one copy covers both halves (engine cost is free-size bound);
    # ACT has slack here, DVE is the tail-critical engine
    nc.scalar.copy(
        out=e_pn[:, :, :],
        in_=m_all[0:64, 0, 32:40].unsqueeze(1).broadcast_to([64, 32, 8]))
    for hh in range(2):
        nc.sync.dma_start(
            out=out_pn_dev[hh * 32:hh * 32 + 32, :, :],
            in_=e_pn[hh * 32:hh * 32 + 32, :, :].rearrange("p a b -> p (a b)")
                .unsqueeze(1).broadcast_to([32, 32, 256]))

    # ---- ps: score chunks + pipelined softmax (per-chunk tiles) ----
    MUL_ENG = ['vector', 'vector', 'gpsimd', 'gpsimd', 'vector', 'vector']

    def ps_soft(ci):
        a, b = CHUNKS[ci]
        n = b - a
        nc.scalar.activation(out=p_c[ci][:, :, :, :], in_=big[ci][:, :, :, :],
                             func=mybir.ActivationFunctionType.Exp,
                             scale=SCALE)
        nc.vector.reduce_sum(out=rs_c[ci][:, :, :], in_=p_c[ci][:, :, :, :],
                             axis=mybir.AxisListType.X)
        nc.vector.reciprocal(out=rr_c[ci][:, :, :], in_=rs_c[ci][:, :, :])
        getattr(nc, MUL_ENG[ci]).tensor_tensor(
            out=pd_c[ci][:, :, :, :], in0=p_c[ci][:, :, :, :],
            in1=rr_c[ci][:, :, :].unsqueeze(3).broadcast_to([128, 4, n, 32]),
            op=mybir.AluOpType.mult)

    ps_soft(0)
    ps_scores(2)
    ps_soft(1)
    ps_scores(3)
    ps_soft(2)
    ps_scores(4)
    ps_scores(5)
    ps_soft(3)
    ps_soft(4)
    ps_soft(5)

    # ---- ps channel-mean: PSUM-accumulated sel matmuls ----
    # each accumulation group's 16 matmuls are contiguous (hp-major)
    for hp in (0, 2, 1, 3):   # group equal tile_position cols
        g, gh = divmod(hp, 2)
        for ci, (a, b) in enumerate(CHUNKS):
            for cp in range(a, b):
                nc.tensor.matmul(
                    m_all[gh * 64:gh * 64 + 64, g, 0:32], sel_ps[:, :],
                    pd_c[ci][:, hp, cp - a, :],
                    start=(cp == 0), stop=(cp == 15),
                    tile_position=(0, gh * 64))

    # ---- ps expand + output DMAs ----
    # one copy per g (ACT & DVE in parallel; engine cost is free-size bound),
    # then per-head-pair DMAs so the DMA engines start on the first half
    nc.scalar.copy(
        out=e_ps[0][:, :, :],
        in_=m_all[:, 0, 0:32].unsqueeze(2).broadcast_to([128, 32, 8]))
    nc.vector.tensor_copy(
        out=e_ps[1][:, :, :],
        in_=m_all[:, 1, 0:32].unsqueeze(2).broadcast_to([128, 32, 8]))
    for g in range(2):
        for gh in range(2):
            nc.sync.dma_start(
                out=out_ps_dev[g, gh * 64:gh * 64 + 64, :, :],
                in_=e_ps[g][gh * 64:gh * 64 + 64, :, :]
                    .rearrange("p a b -> p (a b)")
                    .unsqueeze(1).broadcast_to([64, 8, 256]))

    lowp.__exit__(None, None, None)
    for p in reversed(pools):
        p.__exit__(None, None, None)


@functools.lru_cache(maxsize=1)
def _module():
    nc = bacc.Bacc()
    qt_ps = nc.dram_tensor("qt_ps", [128, 8, 8, 32], FP8, kind="ExternalInput")
    qt_ps2 = nc.dram_tensor("qt_ps2", [128, 8, 8, 32], BF16, kind="ExternalInput")
    kt_ps = nc.dram_tensor("kt_ps", [128, 16, 8, 32], BF16, kind="ExternalInput")
    qt_pn = nc.dram_tensor("qt_pn", [128, 16, 8, 16], FP8, kind="ExternalInput")
    kt_pn = nc.dram_tensor("kt_pn", [128, 16, 8, 8], BF16, kind="ExternalInput")
    out_ps_dev = nc.dram_tensor("out_ps_dev", [2, 128, 8, 256], BF16,
                                kind="ExternalOutput")
    out_pn_dev = nc.dram_tensor("out_pn_dev", [64, 32, 256], BF16,
                                kind="ExternalOutput")
    with tile.TileContext(nc) as tc:
        _body(tc, qt_ps[:, :, :, :], qt_ps2[:, :, :, :], kt_ps[:, :, :, :],
              qt_pn[:, :, :, :], kt_pn[:, :, :, :], out_ps_dev[:, :, :, :],
              out_pn_dev[:, :, :])
    nc.compile()
    return nc


def _pack_ps(a, dt):
    """[32c, 32x, 8h, 64e] f32 -> [128=(cA,e), 16cp, 8h, 32x] in dtype dt"""
    t = a.reshape(16, 2, 32, 8, 64).transpose(1, 4, 0, 3, 2)
    return np.ascontiguousarray(t.reshape(128, 16, 8, 32).astype(dt))


def _pack_pn_q(a, dt):
    """[32c, 8i, 8h, 64e] f32 -> block-diag [128=(cA,e), 16cp, 8h, 16=(cA,i)]"""
    t = a.reshape(16, 2, 8, 8, 64).transpose(1, 4, 0, 3, 2)  # cA,e,cp,h,i
    pad = np.zeros((2, 64, 16, 8, 2, 8), dtype=np.float32)
    pad[0, :, :, :, 0] = t[0]
    pad[1, :, :, :, 1] = t[1]
    return np.ascontiguousarray(pad.reshape(128, 16, 8, 16).astype(dt))


def _pack_pn_k(a, dt):
    """[32c, 8s, 8h, 64e] f32 -> [128=(cA,e), 16cp, 8h, 8s]"""
    t = a.reshape(16, 2, 8, 8, 64).transpose(1, 4, 0, 3, 2)
    return np.ascontiguousarray(t.reshape(128, 16, 8, 8).astype(dt))


def kernel(queries_patch_size, keys_patch_size, queries_patch_num,
           keys_patch_num, patch_index=0, attn_mask=0, **_ignored):
    import ml_dtypes
    from concourse.bass_utils import run_bass_kernel_spmd

    E4 = ml_dtypes.float8_e4m3
    BF = ml_dtypes.bfloat16
    q_ps = np.asarray(queries_patch_size, dtype=np.float32)
    k_ps = np.asarray(keys_patch_size, dtype=np.float32)
    q_pn = np.asarray(queries_patch_num, dtype=np.float32)
    k_pn = np.asarray(keys_patch_num, dtype=np.float32)

    nc = _module()
    in_maps = []
    for i in range(N_CORES):
        sl = slice(i * CH, (i + 1) * CH)
        in_maps.append({
            "qt_ps": np.ascontiguousarray(_pack_ps(q_ps[sl], E4)[:, 0:8]),
            "qt_ps2": np.ascontiguousarray(
                _pack_ps(q_ps[sl], BF)[:, 8:16]),
            "kt_ps": _pack_ps(k_ps[sl], BF),
            "qt_pn": _pack_pn_q(q_pn[sl], E4),
            "kt_pn": _pack_pn_k(k_pn[sl], BF),
        })
    res = run_bass_kernel_spmd(nc, in_maps, core_ids=list(range(N_CORES)))
    s_ps = np.empty((N_CORES, H, WIN, WIN), dtype=np.float32)
    s_pn = np.empty((N_CORES, H, WIN, WIN), dtype=np.float32)
    for i in range(N_CORES):
        dps = np.asarray(res.results[i]["out_ps_dev"]).astype(np.float32)
        # [2g, 128=(hq,l), 8k, 256c] -> [8h, 256r, 256c], r = l*8+k
        s_ps[i] = dps.reshape(8, 32, 8, 256).reshape(8, 256, 256)
        dpn = np.asarray(res.results[i]["out_pn_dev"]).astype(np.float32)
        # [64=(h,i), 32rb, 256c] -> [8h, 256r, 256c], r = rb*8+i
        s_pn[i] = dpn.reshape(8, 8, 32, 256).transpose(0, 2, 1, 3) \
                     .reshape(8, 256, 256)
    return (s_ps, s_pn)
